# revision 2
# baseline (speedup 1.0000x reference)
"""Trainium2 Bass kernel for the k-mer transformer problem.

Semantics (k=3, one-hot 3-mer filters over 4 bases):
    z[b, c, l] = relu(x[b,0,l,d0] + x[b,0,l+1,d1] + x[b,0,l+2,d2] - 2)
      where c = 16*d0 + 4*d1 + d2,  l in [0, 99999)
    out[b, 0, r*33333 + q, c] = z[b, c, 3q + r]      (mod-3 interleave)

Strategy: pure data parallel (batch elem b -> NeuronCore b). Per core the
output (25.6 MB f32) is produced in the permuted order so every store is a
dense contiguous DMA, and it goes over the wire as bf16 (12.8 MB) — the
harness tolerance (2e-2) dwarfs the ~2e-3 bf16 rounding; the host upcasts.
x is staged bf16 for the same reason (input DMA halved).

The elementwise work is split across three engines so no single engine is
the bottleneck (DVE tensor_tensor has no f32 fast path and runs at ~123
G elem/s; Pool and ACT run ~153.6 G elem/s; DVE tensor_scalar runs 2x):
  - Pool:  t1[t,(d0,d1)] = x[t,d0] + x[t+1,d1], and t2 for e in [8,16)
  - DVE:   t2[t,e,d2] = t1[t,e] + x[t+2,d2] for e in [0,8), plus the
           fused relu (v-2 then max 0, one tensor_scalar pass) for e<4
  - ACT:   relu(t2 - 2) -> bf16 for e in [4,16)
All DMAs ride the SP ring: 1 input load + 7 stores (phase 2 is split
131/87/43 so the non-overlappable final store is short) = 8 <= 8 HWDGE
sem lanes (9+ would add a lane-reuse wait -> walrus "Too many sync wait
commands").

Per-partition layout: partition p owns q in [261*p, 261*(p+1)), i.e. x rows
[783*p, 783*p + 785). The host stages x as a [128, 3160] bf16 array whose
row p is x.flat[3132*p : 3132*p + 3160] (zero padded past the end).
"""

import sys

import ml_dtypes
import numpy as np

sys.path.insert(0, "/opt/trn_rl_repo")

import concourse.bacc as bacc  # noqa: E402
import concourse.mybir as mybir  # noqa: E402
from concourse.bass_utils import run_bass_kernel_spmd  # noqa: E402
from concourse.tile import TileContext  # noqa: E402

P = 128  # SBUF partitions
QP = 261  # q-positions per partition (padded: 128*261 = 33408 >= 33333)
Q = 33333  # valid q-positions per phase (99999 / 3)
GMAX = 131  # largest chunk length (tiles are allocated at this size)
PHASES = [  # per-phase (q-local start, len) chunks
    [(0, 131), (131, 130)],
    [(0, 131), (131, 130)],
    [(0, 131), (131, 87), (218, 43)],
]
E_DVE = 8  # t2 e-groups [0, E_DVE) on DVE, rest on Pool
E_RELU_DVE = 4  # relu e-groups [0, E_RELU_DVE) on DVE, rest on ACT
XW = 3160  # staged elems per partition (>= 12*261 + 8 + 8 + extra)
XSTRIDE = 3132  # elem advance per partition (783 rows * 4 ch)
L = 100001
N_CORES = 8

_CACHE = {}


def _build_bass():
    # Bacc (not raw Bass): its finalize() runs generate_event_semaphores,
    # which splits multi-sem waits (HW allows at most 1 wait per inst).
    nc = bacc.Bacc()
    f32 = mybir.dt.float32
    bf16 = mybir.dt.bfloat16
    add = mybir.AluOpType.add
    mx = mybir.AluOpType.max
    relu = mybir.ActivationFunctionType.Relu

    x_d = nc.declare_dram_parameter("x", [P, XW], bf16, isOutput=False)
    y_d = nc.declare_dram_parameter("y", [3, P, QP * 64], bf16, isOutput=True)

    with TileContext(nc) as tc:
        with (
            tc.tile_pool(name="xp", bufs=1) as xp,
            tc.tile_pool(name="t1p", bufs=2) as t1p,
            tc.tile_pool(name="t2p_", bufs=2) as t2p_,
            tc.tile_pool(name="op_", bufs=2) as op_,
        ):
            x_sb = xp.tile([P, XW], bf16)
            nc.sync.dma_start(out=x_sb, in_=x_d[:])
            bias_sb = xp.tile([P, 1], f32, tag="bias")
            nc.vector.memset(bias_sb, -2.0)
            for r in range(3):
                for g0, G in PHASES[r]:
                    base = 12 * g0 + 4 * r
                    # t1[p, t, a, b] = x[base+12t+a] + x[base+4+12t+b]
                    a_ap = (
                        x_sb[:, base : base + 12 * G]
                        .rearrange("p (t u) -> p t u", u=12)[:, :, 0:4]
                        .broadcast_to([P, G, 4, 4])
                    )
                    b_ap = (
                        x_sb[:, base + 4 : base + 4 + 12 * G]
                        .rearrange("p (t u) -> p t u", u=12)[:, :, 0:4]
                        .unsqueeze(2)
                        .broadcast_to([P, G, 4, 4])
                    )
                    t1 = t1p.tile([P, GMAX * 16], f32, tag="t1")
                    t1v = t1.rearrange("p (t e) -> p t e", e=16)[:, 0:G]
                    nc.gpsimd.tensor_tensor(
                        t1v.rearrange("p t (a b) -> p t a b", b=4),
                        a_ap,
                        b_ap,
                        add,
                    )
                    # t2[p, t, e, b] = t1[t, e] + x[base+8+12t+b], split on e
                    t2 = t2p_.tile([P, GMAX * 64], f32, tag="t2")
                    t2v = t2.rearrange("p (t e b) -> p t e b", e=16, b=4)[
                        :, 0:G
                    ]
                    cv = (
                        x_sb[:, base + 8 : base + 8 + 12 * G]
                        .rearrange("p (t u) -> p t u", u=12)[:, :, 0:4]
                        .unsqueeze(2)
                    )
                    for eng, e0, e1 in (
                        (nc.vector, 0, E_DVE),
                        (nc.gpsimd, E_DVE, 16),
                    ):
                        t1_b = t1v[:, :, e0:e1].broadcast_to(
                            [P, G, e1 - e0, 4]
                        )
                        c_ap = cv.broadcast_to([P, G, e1 - e0, 4])
                        eng.tensor_tensor(t2v[:, :, e0:e1, :], t1_b, c_ap, add)
                    # o = relu(t2 - 2) -> bf16, split on e between DVE
                    # (fused add/max tensor_scalar, 2x mode) and ACT
                    o = op_.tile([P, GMAX * 64], bf16, tag="o")
                    ov = o.rearrange("p (t e b) -> p t e b", e=16, b=4)[:, 0:G]
                    nc.vector.tensor_scalar(
                        ov[:, :, 0:E_RELU_DVE, :],
                        t2v[:, :, 0:E_RELU_DVE, :],
                        -2.0,
                        0.0,
                        add,
                        mx,
                    )
                    nc.scalar.activation(
                        ov[:, :, E_RELU_DVE:16, :],
                        t2v[:, :, E_RELU_DVE:16, :],
                        relu,
                        bias=bias_sb,
                    )
                    nc.sync.dma_start(
                        out=y_d[r, :, g0 * 64 : (g0 + G) * 64],
                        in_=o[:, : G * 64],
                    )
    return nc


def _stage_inputs(x):
    """x: [8, 1, L, 4] f32 -> list of per-core {'x': [P, XW] bf16}."""
    need = XSTRIDE * (P - 1) + XW
    in_maps = []
    for b in range(x.shape[0]):
        xf = np.zeros(need, dtype=np.float32)
        xf[: L * 4] = x[b, 0].ravel()
        xbf = xf.astype(ml_dtypes.bfloat16)
        xs = np.lib.stride_tricks.as_strided(
            xbf, shape=(P, XW), strides=(XSTRIDE * 2, 2)
        )
        in_maps.append({"x": np.ascontiguousarray(xs)})
    return in_maps


def _gather_output(results):
    out = np.empty((len(results), 1, 3 * Q, 64), dtype=np.float32)
    for b, res in enumerate(results):
        y = np.asarray(res["y"]).astype(np.float32)
        y = y.reshape(3, P * QP, 64)[:, :Q, :]
        out[b, 0] = y.reshape(3 * Q, 64)
    return out


def _built_and_finalized():
    if "nc" not in _CACHE:
        nc = _build_bass()
        # run_bass_via_pjrt never finalizes; Bacc.finalize runs the register
        # allocation + sync-wait legalization passes walrus requires.
        nc.finalize()
        _CACHE["nc"] = nc
    return _CACHE["nc"]


def run(x, trace=False):
    nc = _built_and_finalized()
    in_maps = _stage_inputs(np.asarray(x, dtype=np.float32))
    bkr = run_bass_kernel_spmd(nc, in_maps, list(range(N_CORES)), trace=trace)
    return _gather_output(bkr.results), bkr


def kernel(x, W=None):
    out, _ = run(x, trace=False)
    return out


# revision 9
# speedup vs baseline: 1.0818x; 1.0818x over previous
"""Trainium2 Bass kernel for the k-mer transformer problem.

Semantics (k=3, one-hot 3-mer filters over 4 bases):
    z[b, c, l] = relu(x[b,0,l,d0] + x[b,0,l+1,d1] + x[b,0,l+2,d2] - 2)
      where c = 16*d0 + 4*d1 + d2,  l in [0, 99999)
    out[b, 0, r*33333 + q, c] = z[b, c, 3q + r]      (mod-3 interleave)

Strategy: pure data parallel (batch elem b -> NeuronCore b). Per core the
output (25.6 MB f32) is produced in the permuted order so every store is a
dense contiguous DMA, and it goes over the wire as bf16 (12.8 MB) — the
harness tolerance (2e-2) dwarfs the ~2e-3 bf16 rounding; the host upcasts.
x is staged bf16 for the same reason (input DMA halved).

The elementwise work is split across three engines so no single engine is
the bottleneck. Measured rates (ntff profile of earlier variants): DVE
f32 tensor_tensor ~123 G elem/s but ~2x slower with any bf16 operand;
Pool tensor_tensor add runs at 0.42 software efficiency (~64.5 G elem/s);
ACT activation runs ~153.6 G elem/s at any dtype mix. Hence x stays f32
on-chip and the split is:
  - Pool:  t1[t,(d0,d1)] = x[t,d0] + x[t+1,d1], and t2 for e in [13,16)
  - DVE:   t2[t,e,d2] = t1[t,e] + x[t+2,d2] for e in [0,13)
  - ACT:   relu(t2 - 2) -> bf16 for all e (native downcast is free there)
All DMAs ride the SP ring: 1 input load + 7 stores (phase 2 is split
131/87/43 so the non-overlappable final store is short) = 8 <= 8 HWDGE
sem lanes (9+ would add a lane-reuse wait -> walrus "Too many sync wait
commands").

Per-partition layout: partition p owns q in [261*p, 261*(p+1)), i.e. x rows
[783*p, 783*p + 785). The host stages x as a [128, 3160] bf16 array whose
row p is x.flat[3132*p : 3132*p + 3160] (zero padded past the end).
"""

import sys

import ml_dtypes
import numpy as np

sys.path.insert(0, "/opt/trn_rl_repo")

import concourse.bacc as bacc  # noqa: E402
import concourse.mybir as mybir  # noqa: E402
from concourse.bass_utils import run_bass_kernel_spmd  # noqa: E402
from concourse.tile import TileContext  # noqa: E402

P = 128  # SBUF partitions
QP = 261  # q-positions per partition (padded: 128*261 = 33408 >= 33333)
Q = 33333  # valid q-positions per phase (99999 / 3)
GMAX = 131  # largest chunk length (tiles are allocated at this size)
PHASES = [  # per-phase (q-local start, len) chunks
    [(0, 131), (131, 130)],
    [(0, 131), (131, 130)],
    [(0, 131), (131, 87), (218, 43)],
]
E_DVE = 13  # t2 e-groups [0, E_DVE) on DVE, rest on Pool
XW = 3160  # staged elems per partition (>= 12*261 + 8 + 8 + extra)
XSTRIDE = 3132  # elem advance per partition (783 rows * 4 ch)
L = 100001
N_CORES = 8

_CACHE = {}


def _build_bass():
    # Bacc (not raw Bass): its finalize() runs generate_event_semaphores,
    # which splits multi-sem waits (HW allows at most 1 wait per inst).
    nc = bacc.Bacc()
    f32 = mybir.dt.float32
    bf16 = mybir.dt.bfloat16
    add = mybir.AluOpType.add
    relu = mybir.ActivationFunctionType.Relu

    x_d = nc.declare_dram_parameter("x", [P, XW], f32, isOutput=False)
    y_d = nc.declare_dram_parameter("y", [3, P, QP * 64], bf16, isOutput=True)

    with TileContext(nc) as tc:
        with (
            tc.tile_pool(name="xp", bufs=1) as xp,
            tc.tile_pool(name="t1p", bufs=2) as t1p,
            tc.tile_pool(name="t2p_", bufs=2) as t2p_,
            tc.tile_pool(name="op_", bufs=2) as op_,
        ):
            x_sb = xp.tile([P, XW], f32)
            nc.sync.dma_start(out=x_sb, in_=x_d[:])
            bias_sb = xp.tile([P, 1], f32, tag="bias")
            nc.vector.memset(bias_sb, -2.0)
            for r in range(3):
                for g0, G in PHASES[r]:
                    base = 12 * g0 + 4 * r
                    # t1[p, t, a, b] = x[base+12t+a] + x[base+4+12t+b]
                    a_ap = (
                        x_sb[:, base : base + 12 * G]
                        .rearrange("p (t u) -> p t u", u=12)[:, :, 0:4]
                        .broadcast_to([P, G, 4, 4])
                    )
                    b_ap = (
                        x_sb[:, base + 4 : base + 4 + 12 * G]
                        .rearrange("p (t u) -> p t u", u=12)[:, :, 0:4]
                        .unsqueeze(2)
                        .broadcast_to([P, G, 4, 4])
                    )
                    t1 = t1p.tile([P, GMAX * 16], f32, tag="t1")
                    t1v = t1.rearrange("p (t e) -> p t e", e=16)[:, 0:G]
                    nc.gpsimd.tensor_tensor(
                        t1v.rearrange("p t (a b) -> p t a b", b=4),
                        a_ap,
                        b_ap,
                        add,
                    )
                    # t2[p, t, e, b] = t1[t, e] + x[base+8+12t+b], split on e
                    t2 = t2p_.tile([P, GMAX * 64], f32, tag="t2")
                    t2v = t2.rearrange("p (t e b) -> p t e b", e=16, b=4)[
                        :, 0:G
                    ]
                    cv = (
                        x_sb[:, base + 8 : base + 8 + 12 * G]
                        .rearrange("p (t u) -> p t u", u=12)[:, :, 0:4]
                        .unsqueeze(2)
                    )
                    for eng, e0, e1 in (
                        (nc.vector, 0, E_DVE),
                        (nc.gpsimd, E_DVE, 16),
                    ):
                        t1_b = t1v[:, :, e0:e1].broadcast_to(
                            [P, G, e1 - e0, 4]
                        )
                        c_ap = cv.broadcast_to([P, G, e1 - e0, 4])
                        eng.tensor_tensor(t2v[:, :, e0:e1, :], t1_b, c_ap, add)
                    # o = relu(t2 - 2) -> bf16 on ACT (native downcast)
                    o = op_.tile([P, GMAX * 64], bf16, tag="o")
                    nc.scalar.activation(
                        o[:, : G * 64],
                        t2[:, : G * 64],
                        relu,
                        bias=bias_sb,
                    )
                    nc.sync.dma_start(
                        out=y_d[r, :, g0 * 64 : (g0 + G) * 64],
                        in_=o[:, : G * 64],
                    )
    return nc


def _stage_inputs(x):
    """x: [8, 1, L, 4] f32 -> list of per-core {'x': [P, XW] f32}."""
    need = XSTRIDE * (P - 1) + XW
    in_maps = []
    for b in range(x.shape[0]):
        xf = np.zeros(need, dtype=np.float32)
        xf[: L * 4] = x[b, 0].ravel()
        xs = np.lib.stride_tricks.as_strided(
            xf, shape=(P, XW), strides=(XSTRIDE * 4, 4)
        )
        in_maps.append({"x": np.ascontiguousarray(xs)})
    return in_maps


def _gather_output(results):
    out = np.empty((len(results), 1, 3 * Q, 64), dtype=np.float32)
    for b, res in enumerate(results):
        y = np.asarray(res["y"]).astype(np.float32)
        y = y.reshape(3, P * QP, 64)[:, :Q, :]
        out[b, 0] = y.reshape(3 * Q, 64)
    return out


def _built_and_finalized():
    if "nc" not in _CACHE:
        nc = _build_bass()
        # run_bass_via_pjrt never finalizes; Bacc.finalize runs the register
        # allocation + sync-wait legalization passes walrus requires.
        nc.finalize()
        _CACHE["nc"] = nc
    return _CACHE["nc"]


def run(x, trace=False):
    nc = _built_and_finalized()
    in_maps = _stage_inputs(np.asarray(x, dtype=np.float32))
    bkr = run_bass_kernel_spmd(nc, in_maps, list(range(N_CORES)), trace=trace)
    return _gather_output(bkr.results), bkr


def kernel(x, W=None):
    out, _ = run(x, trace=False)
    return out


# revision 18
# speedup vs baseline: 1.2857x; 1.1884x over previous
"""Trainium2 Bass kernel for the k-mer transformer problem.

Semantics (k=3, one-hot 3-mer filters over 4 bases):
    z[l, c] = relu(x[l,d0] + x[l+1,d1] + x[l+2,d2] - 2)
      where c = 16*d0 + 4*d1 + d2,  l in [0, 99999)
    out[b, 0, r*33333 + q, c] = z[3q + r, c]      (mod-3 interleave)

Strategy: pure data parallel (batch elem b -> NeuronCore b), and the conv
is a matmul on the tensor engine (the only engine with headroom: vector
f32 adds cap at ~123 G elem/s, Pool's software tensor_tensor at ~40-65).

Key layout trick: within one phase r the positions l = 3q + r are stride-3,
so each output position consumes 12 *consecutive* x floats
x.flat[12q+4r : 12q+4r+12] -- no input replication. Two position streams
(A: q in [0,16704), B: q+16704) are packed as M=128 output rows
(64 channels x 2 streams), so the PE streams 2 positions per column.
The staged moving tensor XT [24, 3*M] holds one 24-row window per phase in
its column band [r*M, (r+1)*M): row 8t+s of band r is
x.flat[12m + 4(r+t) + s] for s<4 (stream A) and
x.flat[12(m+16704) + 4(r+t) + s-4] for s>=4 (stream B). Matmul operands
must sit at SBUF base partition 0/32/64, so the bands live side by side in
columns (all at base 0) rather than as partition-shifted windows. A single
stationary weight W[24,128] serves all phases; the -2 bias folds into the
relu evict, not the matmul.

PSUM [128, 512] f32 tiles (one bank per matmul, 4 banks per group, 2
groups in flight) are evicted as relu(v-2) -> bf16 by ACT / Pool / DVE in
parallel column slices sized to their measured rates (~153/92/57 G
elem/s). Output rides to HBM as bf16 (12.8 MB/core; harness tolerance
2e-2 dwarfs the ~8e-3 worst-case bf16 path error); the host transposes
[2*64, 16704] -> [q, c] and upcasts during the gather.

DMA budget: 1 XT load + 1 W load + 2 stores x 3 phases = 8 <= 8 HWDGE sem
lanes (9+ adds a lane-reuse wait -> walrus "Too many sync wait commands").
"""

import sys

import ml_dtypes
import numpy as np

sys.path.insert(0, "/opt/trn_rl_repo")

import concourse.bacc as bacc  # noqa: E402
import concourse.mybir as mybir  # noqa: E402
from concourse.bass_utils import run_bass_kernel_spmd  # noqa: E402
from concourse.tile import TileContext  # noqa: E402

P = 128
Q = 33333  # valid q-positions per phase (99999 / 3)
M = 16704  # columns per stream (2 streams: q and q + M; 2*M = 33408 >= Q)
NROW = 24  # XT rows: 3 t-blocks x (4 A-rows + 4 B-rows), one window/phase
GROUP = 2048  # psum group columns (4 banks x 512)
# evict column split per group, proportional to measured engine rates
# (GPSIMD/Pool cannot read PSUM, so only ACT ~153.6 and DVE ~57 G elem/s)
ACT_COLS = 1472  # DVE gets the rest (576)
L = 100001
N_CORES = 8

_CACHE = {}


def _kmer_w():
    """Stationary [24, 128] weights: row 8*jj+s, s<4 -> tap (jj, d=s) of
    stream A (cols 0:64), s>=4 -> tap (jj, d=s-4) of stream B (cols 64:128).
    """
    w = np.zeros((24, 128), dtype=np.float32)
    c = np.arange(64)
    digits = np.stack([c // 16, (c // 4) % 4, c % 4])  # [jj, c]
    for k in range(24):
        jj, s = k // 8, k % 8
        blk, d = (0, s) if s < 4 else (1, s - 4)
        w[k, 64 * blk + c[digits[jj] == d]] = 1.0
    return w.astype(ml_dtypes.bfloat16)


def _build_bass():
    # Bacc (not raw Bass): its finalize() runs generate_event_semaphores,
    # which splits multi-sem waits (HW allows at most 1 wait per inst).
    nc = bacc.Bacc()
    f32 = mybir.dt.float32
    bf16 = mybir.dt.bfloat16
    add = mybir.AluOpType.add
    mx = mybir.AluOpType.max
    relu = mybir.ActivationFunctionType.Relu

    xt_d = nc.declare_dram_parameter("xt", [NROW, 3 * M], bf16, isOutput=False)
    w_d = nc.declare_dram_parameter("w", [24, 128], bf16, isOutput=False)
    y_d = nc.declare_dram_parameter("y", [3, P, M], bf16, isOutput=True)

    with TileContext(nc) as tc:
        with (
            tc.tile_pool(name="xp", bufs=1) as xp,
            tc.tile_pool(name="pp", bufs=2, space="PSUM") as pp,
            tc.tile_pool(name="op_", bufs=2) as op_,
        ):
            xt_sb = xp.tile([NROW, 3 * M], bf16)
            nc.sync.dma_start(out=xt_sb, in_=xt_d[:])
            w_sb = xp.tile([24, 128], bf16, tag="w")
            nc.sync.dma_start(out=w_sb, in_=w_d[:])
            bias_sb = xp.tile([P, 1], f32, tag="bias")
            nc.vector.memset(bias_sb, -2.0)
            for r in range(3):
                o = op_.tile([P, M], bf16, tag="o")
                rhs_rows = xt_sb[:, r * M : (r + 1) * M]
                for g0 in range(0, M, GROUP):
                    gw = min(GROUP, M - g0)
                    ps = pp.tile([P, GROUP], f32, tag="ps")
                    for k0 in range(0, gw, 512):
                        kw = min(512, gw - k0)
                        nc.tensor.matmul(
                            out=ps[:, k0 : k0 + kw],
                            lhsT=w_sb[:],
                            rhs=rhs_rows[:, g0 + k0 : g0 + k0 + kw],
                            start=True,
                            stop=True,
                        )
                    # relu(v - 2) -> bf16; ACT and DVE split the columns
                    sp = (ACT_COLS * gw) // GROUP
                    nc.scalar.activation(
                        o[:, g0 : g0 + sp],
                        ps[:, 0:sp],
                        relu,
                        bias=bias_sb,
                    )
                    nc.vector.tensor_scalar(
                        o[:, g0 + sp : g0 + gw],
                        ps[:, sp:gw],
                        -2.0,
                        0.0,
                        add,
                        mx,
                    )
                half = 4 * GROUP
                nc.sync.dma_start(out=y_d[r, :, 0:half], in_=o[:, 0:half])
                nc.sync.dma_start(out=y_d[r, :, half:M], in_=o[:, half:M])
    return nc


def _stage_inputs(x):
    """x: [8,1,L,4] f32 -> per-core {'xt': [24, 3*M] bf16, 'w': [24,128]}."""
    w = _kmer_w()
    need = 12 * (2 * M - 1) + 28  # last col of band r=2 reads up to here
    in_maps = []
    for b in range(x.shape[0]):
        xf = np.zeros(need, dtype=np.float32)
        xf[: L * 4] = x[b, 0].ravel()
        xt = np.empty((NROW, 3 * M), dtype=np.float32)
        for r in range(3):
            band = xt[:, r * M : (r + 1) * M]
            for t in range(3):
                for s in range(4):
                    band[8 * t + s] = xf[4 * (r + t) + s :: 12][:M]
                    band[8 * t + s + 4] = xf[12 * M + 4 * (r + t) + s :: 12][
                        :M
                    ]
        in_maps.append({"xt": xt.astype(ml_dtypes.bfloat16), "w": w})
    return in_maps


def _gather_output(results):
    out = np.empty((len(results), 1, 3 * Q, 64), dtype=np.float32)
    for b, res in enumerate(results):
        y = np.asarray(res["y"]).astype(np.float32)  # [3, 128, M]
        for r in range(3):
            zr = y[r].reshape(2, 64, M).transpose(0, 2, 1).reshape(2 * M, 64)
            out[b, 0, r * Q : (r + 1) * Q, :] = zr[:Q]
    return out


def _built_and_finalized():
    if "nc" not in _CACHE:
        nc = _build_bass()
        # run_bass_via_pjrt never finalizes; Bacc.finalize runs the register
        # allocation + sync-wait legalization passes walrus requires.
        nc.finalize()
        _CACHE["nc"] = nc
    return _CACHE["nc"]


def run(x, trace=False):
    nc = _built_and_finalized()
    in_maps = _stage_inputs(np.asarray(x, dtype=np.float32))
    bkr = run_bass_kernel_spmd(nc, in_maps, list(range(N_CORES)), trace=trace)
    return _gather_output(bkr.results), bkr


def kernel(x, W=None):
    out, _ = run(x, trace=False)
    return out


# revision 24
# speedup vs baseline: 1.3042x; 1.0144x over previous
"""Trainium2 Bass kernel for the k-mer transformer problem.

Semantics (k=3, one-hot 3-mer filters over 4 bases):
    z[l, c] = relu(x[l,d0] + x[l+1,d1] + x[l+2,d2] - 2)
      where c = 16*d0 + 4*d1 + d2,  l in [0, 99999)
    out[b, 0, r*33333 + q, c] = z[3q + r, c]      (mod-3 interleave)

Strategy: pure data parallel (batch elem b -> NeuronCore b), and the conv
is a matmul on the tensor engine (the only engine with headroom: vector
f32 adds cap at ~123 G elem/s, Pool's software tensor_tensor at ~40-65).

Key layout trick: within one phase r the positions l = 3q + r are stride-3,
so each output position consumes 12 *consecutive* x floats
x.flat[12q+4r : 12q+4r+12] -- no input replication. Two position streams
(A: q in [0,16704), B: q+16704) are packed as M=128 output rows
(64 channels x 2 streams), so the PE streams 2 positions per column.
The staged moving tensor holds one 24-row window per phase: row 8t+s of
phase r's window is x.flat[12m + 4(r+t) + s] for s<4 (stream A) and
x.flat[12(m+16704) + 4(r+t) + s-4] for s>=4 (stream B). Matmul operands
must sit at SBUF base partition 0/32/64, and DMA into few partitions is
slow (per-partition SBUF write bandwidth), so the windows are spread over
partitions: xtA [56, M+128] carries phase 0 at rows 0:24 and phase 1 at
rows 32:56 (plus the stationary W[24,128] in its last 128 columns), and
xtB [24, M] carries phase 2 at rows 0:24. A single W serves all phases
(the window structure is phase-invariant); the -2 bias folds into the
relu evict, not the matmul.

PSUM [128, 512] f32 tiles (one bank per matmul, 4 banks per group, 2
groups in flight) are evicted as relu(v-2) -> bf16 by ACT / Pool / DVE in
parallel column slices sized to their measured rates (~153/92/57 G
elem/s). Output rides to HBM as bf16 (12.8 MB/core; harness tolerance
2e-2 dwarfs the ~8e-3 worst-case bf16 path error); the host transposes
[2*64, 16704] -> [q, c] and upcasts during the gather.

DMA budget: 1 XT load + 1 W load + 2 stores x 3 phases = 8 <= 8 HWDGE sem
lanes (9+ adds a lane-reuse wait -> walrus "Too many sync wait commands").
"""

import sys

import ml_dtypes
import numpy as np

sys.path.insert(0, "/opt/trn_rl_repo")

import concourse.bacc as bacc  # noqa: E402
import concourse.mybir as mybir  # noqa: E402
from concourse.bass_utils import run_bass_kernel_spmd  # noqa: E402
from concourse.tile import TileContext  # noqa: E402

P = 128
Q = 33333  # valid q-positions per phase (99999 / 3)
M = 16704  # columns per stream (2 streams: q and q + M; 2*M = 33408 >= Q)
NROW = 24  # XT rows: 3 t-blocks x (4 A-rows + 4 B-rows), one window/phase
GROUP = 2048  # psum group columns (4 banks x 512)
# evict column split per group, proportional to measured engine rates
# (GPSIMD/Pool cannot read PSUM; ACT ~1.20 GHz and DVE ~0.82 GHz observed)
ACT_COLS = 1216  # DVE gets the rest (832)
STORE_A = 6 * GROUP  # uneven store split so the trailing store is short
L = 100001
N_CORES = 8

_CACHE = {}


def _kmer_w():
    """Stationary [24, 128] weights: row 8*jj+s, s<4 -> tap (jj, d=s) of
    stream A (cols 0:64), s>=4 -> tap (jj, d=s-4) of stream B (cols 64:128).
    """
    w = np.zeros((24, 128), dtype=np.float32)
    c = np.arange(64)
    digits = np.stack([c // 16, (c // 4) % 4, c % 4])  # [jj, c]
    for k in range(24):
        jj, s = k // 8, k % 8
        blk, d = (0, s) if s < 4 else (1, s - 4)
        w[k, 64 * blk + c[digits[jj] == d]] = 1.0
    return w.astype(ml_dtypes.bfloat16)


def _build_bass():
    # Bacc (not raw Bass): its finalize() runs generate_event_semaphores,
    # which splits multi-sem waits (HW allows at most 1 wait per inst).
    nc = bacc.Bacc()
    f32 = mybir.dt.float32
    bf16 = mybir.dt.bfloat16
    add = mybir.AluOpType.add
    mx = mybir.AluOpType.max
    relu = mybir.ActivationFunctionType.Relu

    xta_d = nc.declare_dram_parameter("xta", [56, M + 128], bf16, isOutput=False)
    xtb_d = nc.declare_dram_parameter("xtb", [24, M], bf16, isOutput=False)
    y_d = nc.declare_dram_parameter("y", [3, P, M], bf16, isOutput=True)

    with TileContext(nc) as tc:
        with (
            tc.tile_pool(name="xp", bufs=1) as xp,
            tc.tile_pool(name="pp", bufs=2, space="PSUM") as pp,
            tc.tile_pool(name="op_", bufs=2) as op_,
        ):
            xta_sb = xp.tile([56, M + 128], bf16)
            nc.sync.dma_start(out=xta_sb, in_=xta_d[:])
            xtb_sb = xp.tile([24, M], bf16, tag="xtb")
            nc.sync.dma_start(out=xtb_sb, in_=xtb_d[:])
            bias_sb = xp.tile([P, 1], f32, tag="bias")
            nc.vector.memset(bias_sb, -2.0)
            for r in range(3):
                o = op_.tile([P, M], bf16, tag="o")
                # lhsT and rhs must share a base partition (0/32/64)
                rhs_rows = (
                    xta_sb[32 * r : 32 * r + 24, 0:M] if r < 2 else xtb_sb[:]
                )
                w_ap = xta_sb[32 * r : 32 * r + 24, M : M + 128] if r < 2 else xta_sb[0:24, M : M + 128]
                for g0 in range(0, M, GROUP):
                    gw = min(GROUP, M - g0)
                    ps = pp.tile([P, GROUP], f32, tag="ps")
                    for k0 in range(0, gw, 512):
                        kw = min(512, gw - k0)
                        nc.tensor.matmul(
                            out=ps[:, k0 : k0 + kw],
                            lhsT=w_ap,
                            rhs=rhs_rows[:, g0 + k0 : g0 + k0 + kw],
                            start=True,
                            stop=True,
                        )
                    # relu(v - 2) -> bf16; ACT and DVE split the columns
                    sp = (ACT_COLS * gw) // GROUP
                    nc.scalar.activation(
                        o[:, g0 : g0 + sp],
                        ps[:, 0:sp],
                        relu,
                        bias=bias_sb,
                    )
                    nc.vector.tensor_scalar(
                        o[:, g0 + sp : g0 + gw],
                        ps[:, sp:gw],
                        -2.0,
                        0.0,
                        add,
                        mx,
                    )
                nc.sync.dma_start(out=y_d[r, :, 0:STORE_A], in_=o[:, 0:STORE_A])
                nc.sync.dma_start(out=y_d[r, :, STORE_A:M], in_=o[:, STORE_A:M])
    return nc


def _stage_inputs(x):
    """x: [8,1,L,4] f32 -> per-core {'xta': [56, M+128], 'xtb': [24, M]}."""
    w = _kmer_w()
    need = 12 * (2 * M - 1) + 28  # last col of the r=2 window reads up to here
    in_maps = []
    for b in range(x.shape[0]):
        xf = np.zeros(need, dtype=np.float32)
        xf[: L * 4] = x[b, 0].ravel()

        def band(r):
            out = np.empty((NROW, M), dtype=np.float32)
            for t in range(3):
                for s in range(4):
                    out[8 * t + s] = xf[4 * (r + t) + s :: 12][:M]
                    out[8 * t + s + 4] = xf[12 * M + 4 * (r + t) + s :: 12][:M]
            return out

        xta = np.zeros((56, M + 128), dtype=ml_dtypes.bfloat16)
        xta[0:24, 0:M] = band(0).astype(ml_dtypes.bfloat16)
        xta[32:56, 0:M] = band(1).astype(ml_dtypes.bfloat16)
        xta[0:24, M : M + 128] = w  # one W copy per base partition
        xta[32:56, M : M + 128] = w
        xtb = band(2).astype(ml_dtypes.bfloat16)
        in_maps.append({"xta": xta, "xtb": xtb})
    return in_maps


def _gather_output(results):
    out = np.empty((len(results), 1, 3 * Q, 64), dtype=np.float32)
    for b, res in enumerate(results):
        y = np.asarray(res["y"]).astype(np.float32)  # [3, 128, M]
        for r in range(3):
            zr = y[r].reshape(2, 64, M).transpose(0, 2, 1).reshape(2 * M, 64)
            out[b, 0, r * Q : (r + 1) * Q, :] = zr[:Q]
    return out


def _built_and_finalized():
    if "nc" not in _CACHE:
        nc = _build_bass()
        # run_bass_via_pjrt never finalizes; Bacc.finalize runs the register
        # allocation + sync-wait legalization passes walrus requires.
        nc.finalize()
        _CACHE["nc"] = nc
    return _CACHE["nc"]


def run(x, trace=False):
    nc = _built_and_finalized()
    in_maps = _stage_inputs(np.asarray(x, dtype=np.float32))
    bkr = run_bass_kernel_spmd(nc, in_maps, list(range(N_CORES)), trace=trace)
    return _gather_output(bkr.results), bkr


def kernel(x, W=None):
    out, _ = run(x, trace=False)
    return out


# revision 27
# speedup vs baseline: 1.5616x; 1.1974x over previous
"""Trainium2 Bass kernel for the k-mer transformer problem.

Semantics (k=3, one-hot 3-mer filters over 4 bases):
    z[l, c] = relu(x[l,d0] + x[l+1,d1] + x[l+2,d2] - 2)
      where c = 16*d0 + 4*d1 + d2,  l in [0, 99999)
    out[b, 0, r*33333 + q, c] = z[3q + r, c]      (mod-3 interleave)

Strategy: pure data parallel (batch elem b -> NeuronCore b), and the conv
is a matmul on the tensor engine (the only engine with headroom: vector
f32 adds cap at ~123 G elem/s, Pool's software tensor_tensor at ~40-65).

Key layout trick: within one phase r the positions l = 3q + r are stride-3,
so each output position consumes 12 *consecutive* x floats
x.flat[12q+4r : 12q+4r+12] -- no input replication. Two position streams
(A: q in [0,16704), B: q+16704) are packed as M=128 output rows
(64 channels x 2 streams), so the PE streams 2 positions per column.
The staged moving tensor holds one 24-row window per phase: row 8t+s of
phase r's window is x.flat[12m + 4(r+t) + s] for s<4 (stream A) and
x.flat[12(m+16704) + 4(r+t) + s-4] for s>=4 (stream B). Matmul operands
must sit at SBUF base partition 0/32/64, and DMA into few partitions is
slow (per-partition SBUF write bandwidth), so the windows are spread over
partitions: xtA [56, M+128] carries phase 0 at rows 0:24 and phase 1 at
rows 32:56 (plus the stationary W[24,128] in its last 128 columns), and
xtB [24, M] carries phase 2 at rows 0:24. A single W serves all phases
(the window structure is phase-invariant); the -2 bias folds into the
relu evict, not the matmul.

PSUM [128, 512] f32 tiles (one bank per matmul, 4 banks per group, 2
groups in flight) are evicted as relu(v-2) -> bf16 by ACT / Pool / DVE in
parallel column slices sized to their measured rates (~153/92/57 G
elem/s). Output rides to HBM as bf16 (12.8 MB/core; harness tolerance
2e-2 dwarfs the ~8e-3 worst-case bf16 path error); the host transposes
[2*64, 16704] -> [q, c] and upcasts during the gather.

DMA budget: 1 XT load + 1 W load + 2 stores x 3 phases = 8 <= 8 HWDGE sem
lanes (9+ adds a lane-reuse wait -> walrus "Too many sync wait commands").
"""

import sys

import ml_dtypes
import numpy as np

sys.path.insert(0, "/opt/trn_rl_repo")

import concourse.bacc as bacc  # noqa: E402
import concourse.mybir as mybir  # noqa: E402
from concourse.bass_utils import run_bass_kernel_spmd  # noqa: E402
from concourse.tile import TileContext  # noqa: E402

P = 128
Q = 33333  # valid q-positions per phase (99999 / 3)
M = 16704  # columns per stream (2 streams: q and q + M; 2*M = 33408 >= Q)
NROW = 24  # XT rows: 3 t-blocks x (4 A-rows + 4 B-rows), one window/phase
GROUP = 1024  # psum group columns (2 banks x 512); 4 groups in flight
# evict column split per group, proportional to measured engine rates
# (GPSIMD/Pool cannot read PSUM; ACT ~1.03 GHz and DVE ~0.89 GHz effective)
ACT_COLS = 608  # DVE gets the rest (416)
STORE_A = 12 * GROUP  # uneven store split so the trailing store is short
DMA_CHUNK = 8192  # elems per DMA descriptor: >16KB/partition loads run at
# half engine rate, so split load descriptors to 16KB
L = 100001
N_CORES = 8

_CACHE = {}


def _kmer_w():
    """Stationary [24, 128] weights: row 8*jj+s, s<4 -> tap (jj, d=s) of
    stream A (cols 0:64), s>=4 -> tap (jj, d=s-4) of stream B (cols 64:128).
    """
    w = np.zeros((24, 128), dtype=np.float32)
    c = np.arange(64)
    digits = np.stack([c // 16, (c // 4) % 4, c % 4])  # [jj, c]
    for k in range(24):
        jj, s = k // 8, k % 8
        blk, d = (0, s) if s < 4 else (1, s - 4)
        w[k, 64 * blk + c[digits[jj] == d]] = 1.0
    return w.astype(ml_dtypes.bfloat16)


def _build_bass():
    # Bacc (not raw Bass): its finalize() runs generate_event_semaphores,
    # which splits multi-sem waits (HW allows at most 1 wait per inst).
    nc = bacc.Bacc()
    f32 = mybir.dt.float32
    bf16 = mybir.dt.bfloat16
    add = mybir.AluOpType.add
    mx = mybir.AluOpType.max
    relu = mybir.ActivationFunctionType.Relu

    xta_d = nc.declare_dram_parameter("xta", [56, M + 128], bf16, isOutput=False)
    xtb_d = nc.declare_dram_parameter("xtb", [24, M], bf16, isOutput=False)
    y_d = nc.declare_dram_parameter("y", [3, P, M], bf16, isOutput=True)

    with TileContext(nc) as tc:
        with (
            tc.tile_pool(name="xp", bufs=1) as xp,
            tc.tile_pool(name="pp", bufs=4, space="PSUM") as pp,
            tc.tile_pool(name="op_", bufs=2) as op_,
        ):
            xta_sb = xp.tile([56, M + 128], bf16)
            nc.sync.dma_start(
                out=xta_sb, in_=xta_d[:], max_dma_last_dim=DMA_CHUNK
            )
            xtb_sb = xp.tile([24, M], bf16, tag="xtb")
            nc.sync.dma_start(
                out=xtb_sb, in_=xtb_d[:], max_dma_last_dim=DMA_CHUNK
            )
            bias_sb = xp.tile([P, 1], f32, tag="bias")
            nc.vector.memset(bias_sb, -2.0)
            for r in range(3):
                o = op_.tile([P, M], bf16, tag="o")
                # lhsT and rhs must share a base partition (0/32/64)
                rhs_rows = (
                    xta_sb[32 * r : 32 * r + 24, 0:M] if r < 2 else xtb_sb[:]
                )
                w_ap = xta_sb[32 * r : 32 * r + 24, M : M + 128] if r < 2 else xta_sb[0:24, M : M + 128]
                for g0 in range(0, M, GROUP):
                    gw = min(GROUP, M - g0)
                    ps = pp.tile([P, GROUP], f32, tag="ps")
                    for k0 in range(0, gw, 512):
                        kw = min(512, gw - k0)
                        nc.tensor.matmul(
                            out=ps[:, k0 : k0 + kw],
                            lhsT=w_ap,
                            rhs=rhs_rows[:, g0 + k0 : g0 + k0 + kw],
                            start=True,
                            stop=True,
                        )
                    # relu(v - 2) -> bf16; ACT and DVE split the columns
                    sp = (ACT_COLS * gw) // GROUP
                    nc.scalar.activation(
                        o[:, g0 : g0 + sp],
                        ps[:, 0:sp],
                        relu,
                        bias=bias_sb,
                    )
                    nc.vector.tensor_scalar(
                        o[:, g0 + sp : g0 + gw],
                        ps[:, sp:gw],
                        -2.0,
                        0.0,
                        add,
                        mx,
                    )
                nc.sync.dma_start(out=y_d[r, :, 0:STORE_A], in_=o[:, 0:STORE_A])
                nc.sync.dma_start(out=y_d[r, :, STORE_A:M], in_=o[:, STORE_A:M])
    return nc


def _stage_inputs(x):
    """x: [8,1,L,4] f32 -> per-core {'xta': [56, M+128], 'xtb': [24, M]}."""
    w = _kmer_w()
    need = 12 * (2 * M - 1) + 28  # last col of the r=2 window reads up to here
    in_maps = []
    for b in range(x.shape[0]):
        xf = np.zeros(need, dtype=np.float32)
        xf[: L * 4] = x[b, 0].ravel()

        def band(r):
            out = np.empty((NROW, M), dtype=np.float32)
            for t in range(3):
                for s in range(4):
                    out[8 * t + s] = xf[4 * (r + t) + s :: 12][:M]
                    out[8 * t + s + 4] = xf[12 * M + 4 * (r + t) + s :: 12][:M]
            return out

        xta = np.zeros((56, M + 128), dtype=ml_dtypes.bfloat16)
        xta[0:24, 0:M] = band(0).astype(ml_dtypes.bfloat16)
        xta[32:56, 0:M] = band(1).astype(ml_dtypes.bfloat16)
        xta[0:24, M : M + 128] = w  # one W copy per base partition
        xta[32:56, M : M + 128] = w
        xtb = band(2).astype(ml_dtypes.bfloat16)
        in_maps.append({"xta": xta, "xtb": xtb})
    return in_maps


def _gather_output(results):
    out = np.empty((len(results), 1, 3 * Q, 64), dtype=np.float32)
    for b, res in enumerate(results):
        y = np.asarray(res["y"]).astype(np.float32)  # [3, 128, M]
        for r in range(3):
            zr = y[r].reshape(2, 64, M).transpose(0, 2, 1).reshape(2 * M, 64)
            out[b, 0, r * Q : (r + 1) * Q, :] = zr[:Q]
    return out


def _built_and_finalized():
    if "nc" not in _CACHE:
        nc = _build_bass()
        # run_bass_via_pjrt never finalizes; Bacc.finalize runs the register
        # allocation + sync-wait legalization passes walrus requires.
        nc.finalize()
        _CACHE["nc"] = nc
    return _CACHE["nc"]


def run(x, trace=False):
    nc = _built_and_finalized()
    in_maps = _stage_inputs(np.asarray(x, dtype=np.float32))
    bkr = run_bass_kernel_spmd(nc, in_maps, list(range(N_CORES)), trace=trace)
    return _gather_output(bkr.results), bkr


def kernel(x, W=None):
    out, _ = run(x, trace=False)
    return out


# revision 36
# speedup vs baseline: 1.5690x; 1.0047x over previous
"""Trainium2 Bass kernel for the k-mer transformer problem.

Semantics (k=3, one-hot 3-mer filters over 4 bases):
    z[l, c] = relu(x[l,d0] + x[l+1,d1] + x[l+2,d2] - 2)
      where c = 16*d0 + 4*d1 + d2,  l in [0, 99999)
    out[b, 0, r*33333 + q, c] = z[3q + r, c]      (mod-3 interleave)

Strategy: pure data parallel (batch elem b -> NeuronCore b), and the conv
is a matmul on the tensor engine (the only engine with headroom: vector
f32 adds cap at ~123 G elem/s, Pool's software tensor_tensor at ~40-65).

Key layout trick: within one phase r the positions l = 3q + r are stride-3,
so each output position consumes 12 *consecutive* x floats
x.flat[12q+4r : 12q+4r+12] -- no input replication. Two position streams
(A: q in [0,16704), B: q+16704) are packed as M=128 output rows
(64 channels x 2 streams), so the PE streams 2 positions per column.
The staged moving tensor holds one 24-row window per phase: row 8t+s of
phase r's window is x.flat[12m + 4(r+t) + s] for s<4 (stream A) and
x.flat[12(m+16704) + 4(r+t) + s-4] for s>=4 (stream B). Matmul operands
must sit at SBUF base partition 0/32/64, and DMA into few partitions is
slow (per-partition SBUF write bandwidth), so the windows are spread over
partitions: xtA [56, M+128] carries phase 0 at rows 0:24 and phase 1 at
rows 32:56 (plus the stationary W[24,128] in its last 128 columns), and
xtB [24, M] carries phase 2 at rows 0:24. A single W serves all phases
(the window structure is phase-invariant); the -2 bias folds into the
relu evict, not the matmul.

PSUM [128, 512] f32 tiles (one bank per matmul, 4 banks per group, 2
groups in flight) are evicted as relu(v-2) -> bf16 by ACT / Pool / DVE in
parallel column slices sized to their measured rates (~153/92/57 G
elem/s). Output rides to HBM as bf16 (12.8 MB/core; harness tolerance
2e-2 dwarfs the ~8e-3 worst-case bf16 path error); the host transposes
[2*64, 16704] -> [q, c] and upcasts during the gather.

DMA budget: 1 XT load + 1 W load + 2 stores x 3 phases = 8 <= 8 HWDGE sem
lanes (9+ adds a lane-reuse wait -> walrus "Too many sync wait commands").
"""

import sys

import ml_dtypes
import numpy as np

sys.path.insert(0, "/opt/trn_rl_repo")

import concourse.bacc as bacc  # noqa: E402
import concourse.mybir as mybir  # noqa: E402
from concourse.bass_utils import run_bass_kernel_spmd  # noqa: E402
from concourse.tile import TileContext  # noqa: E402

P = 128
Q = 33333  # valid q-positions per phase (99999 / 3)
M = 16704  # columns per stream (2 streams: q and q + M; 2*M = 33408 >= Q)
NROW = 24  # XT rows: 3 t-blocks x (4 A-rows + 4 B-rows), one window/phase
GROUP = 1024  # psum group columns (2 banks x 512); 4 groups in flight
# evict column split per group, proportional to measured engine rates
# (GPSIMD/Pool cannot read PSUM; ACT ~1.03 GHz and DVE ~0.89 GHz effective)
ACT_COLS = 608  # DVE gets the rest (416)
STORE_A = 10 * GROUP  # uneven store split so the trailing store is short
# 33KB/partition load descriptors run at half DMA-engine rate, and in-place
# splits coalesce back into one big descriptor. So each staged row is two
# 8512-element segments separated by a 64-element SBUF gap (non-adjacent ->
# no coalescing; ~17KB descriptors run at full rate). Segment layout, in
# "physical" row coordinates:
#   seg0 @ [0, 8512):      [W: 128][data cols 0..8192][unused 192]
#   seg1 @ [8576, 17088):  [data cols 8192..16704]
# The 8192 boundary is a psum-group multiple so no matmul slice crosses it.
SEG = 8512
SEGSTRIDE = SEG + 64  # 8576
ROWW = 2 * SEGSTRIDE  # 17152 elements per staged SBUF row


def _phys(d):
    """Physical column of logical data column d."""
    return 128 + d if d < 8 * GROUP else SEGSTRIDE + (d - 8 * GROUP)
L = 100001
N_CORES = 8

_CACHE = {}


def _kmer_w():
    """Stationary [24, 128] weights: row 8*jj+s, s<4 -> tap (jj, d=s) of
    stream A (cols 0:64), s>=4 -> tap (jj, d=s-4) of stream B (cols 64:128).
    """
    w = np.zeros((24, 128), dtype=np.float32)
    c = np.arange(64)
    digits = np.stack([c // 16, (c // 4) % 4, c % 4])  # [jj, c]
    for k in range(24):
        jj, s = k // 8, k % 8
        blk, d = (0, s) if s < 4 else (1, s - 4)
        w[k, 64 * blk + c[digits[jj] == d]] = 1.0
    return w.astype(ml_dtypes.bfloat16)


def _build_bass():
    # Bacc (not raw Bass): its finalize() runs generate_event_semaphores,
    # which splits multi-sem waits (HW allows at most 1 wait per inst).
    nc = bacc.Bacc()
    f32 = mybir.dt.float32
    bf16 = mybir.dt.bfloat16
    add = mybir.AluOpType.add
    mx = mybir.AluOpType.max
    relu = mybir.ActivationFunctionType.Relu

    xta_d = nc.declare_dram_parameter("xta", [56, 2, SEG], bf16, isOutput=False)
    xtb_d = nc.declare_dram_parameter("xtb", [24, 2, SEG], bf16, isOutput=False)
    y_d = nc.declare_dram_parameter("y", [3, P, M], bf16, isOutput=True)

    with TileContext(nc) as tc:
        with (
            tc.tile_pool(name="xp", bufs=1) as xp,
            tc.tile_pool(name="pp", bufs=4, space="PSUM") as pp,
            tc.tile_pool(name="op_", bufs=2) as op_,
        ):
            xta_sb = xp.tile([56, ROWW], bf16)
            nc.sync.dma_start(
                out=xta_sb.rearrange("p (c g) -> p c g", c=2)[:, :, 0:SEG],
                in_=xta_d[:],
            )
            xtb_sb = xp.tile([24, ROWW], bf16, tag="xtb")
            nc.sync.dma_start(
                out=xtb_sb.rearrange("p (c g) -> p c g", c=2)[:, :, 0:SEG],
                in_=xtb_d[:],
            )
            bias_sb = xp.tile([P, 1], f32, tag="bias")
            nc.vector.memset(bias_sb, -2.0)
            for r in range(3):
                o = op_.tile([P, M], bf16, tag="o")
                # lhsT and rhs must share a base partition (0/32/64)
                xt, p0 = (xta_sb, 32 * r) if r < 2 else (xtb_sb, 0)
                rhs_rows = xt[p0 : p0 + 24]
                w_ap = xta_sb[p0 : p0 + 24, 0:128] if r < 2 else xtb_sb[0:24, 0:128]
                for g0 in range(0, M, GROUP):
                    gw = min(GROUP, M - g0)
                    ps = pp.tile([P, GROUP], f32, tag="ps")
                    for k0 in range(0, gw, 512):
                        kw = min(512, gw - k0)
                        c0 = _phys(g0 + k0)
                        nc.tensor.matmul(
                            out=ps[:, k0 : k0 + kw],
                            lhsT=w_ap,
                            rhs=rhs_rows[:, c0 : c0 + kw],
                            start=True,
                            stop=True,
                        )
                    # relu(v - 2) -> bf16; ACT and DVE split the columns
                    sp = (ACT_COLS * gw) // GROUP
                    nc.scalar.activation(
                        o[:, g0 : g0 + sp],
                        ps[:, 0:sp],
                        relu,
                        bias=bias_sb,
                    )
                    nc.vector.tensor_scalar(
                        o[:, g0 + sp : g0 + gw],
                        ps[:, sp:gw],
                        -2.0,
                        0.0,
                        add,
                        mx,
                    )
                nc.sync.dma_start(out=y_d[r, :, 0:STORE_A], in_=o[:, 0:STORE_A])
                nc.sync.dma_start(out=y_d[r, :, STORE_A:M], in_=o[:, STORE_A:M])
    return nc


def _stage_inputs(x):
    """x: [8,1,L,4] f32 -> per-core {'xta': [56, M+128], 'xtb': [24, M]}."""
    w = _kmer_w()
    need = 12 * (2 * M - 1) + 28  # last col of the r=2 window reads up to here
    in_maps = []
    for b in range(x.shape[0]):
        xf = np.zeros(need, dtype=np.float32)
        xf[: L * 4] = x[b, 0].ravel()

        def band(r):
            out = np.empty((NROW, M), dtype=np.float32)
            for t in range(3):
                for s in range(4):
                    out[8 * t + s] = xf[4 * (r + t) + s :: 12][:M]
                    out[8 * t + s + 4] = xf[12 * M + 4 * (r + t) + s :: 12][:M]
            return out

        def segs(rows, data, wmat):
            """Pack [W|data 0:8192|pad] and [data 8192:M] segment pairs."""
            out = np.zeros((rows, 2, SEG), dtype=ml_dtypes.bfloat16)
            out[: wmat.shape[0], 0, 0:128] = wmat
            out[: data.shape[0], 0, 128 : 128 + 8 * GROUP] = data[
                :, : 8 * GROUP
            ]
            out[: data.shape[0], 1, : M - 8 * GROUP] = data[:, 8 * GROUP :]
            return out

        b16 = ml_dtypes.bfloat16
        xta = np.zeros((56, 2, SEG), dtype=b16)
        xta[0:24] = segs(24, band(0).astype(b16), w)
        xta[32:56] = segs(24, band(1).astype(b16), w)
        xtb = segs(24, band(2).astype(b16), w)
        in_maps.append({"xta": xta, "xtb": xtb})
    return in_maps


def _gather_output(results):
    out = np.empty((len(results), 1, 3 * Q, 64), dtype=np.float32)
    for b, res in enumerate(results):
        y = np.asarray(res["y"]).astype(np.float32)  # [3, 128, M]
        for r in range(3):
            zr = y[r].reshape(2, 64, M).transpose(0, 2, 1).reshape(2 * M, 64)
            out[b, 0, r * Q : (r + 1) * Q, :] = zr[:Q]
    return out


def _built_and_finalized():
    if "nc" not in _CACHE:
        nc = _build_bass()
        # run_bass_via_pjrt never finalizes; Bacc.finalize runs the register
        # allocation + sync-wait legalization passes walrus requires.
        nc.finalize()
        _CACHE["nc"] = nc
    return _CACHE["nc"]


def run(x, trace=False):
    nc = _built_and_finalized()
    in_maps = _stage_inputs(np.asarray(x, dtype=np.float32))
    bkr = run_bass_kernel_spmd(nc, in_maps, list(range(N_CORES)), trace=trace)
    return _gather_output(bkr.results), bkr


def kernel(x, W=None):
    out, _ = run(x, trace=False)
    return out


# revision 42
# speedup vs baseline: 1.5923x; 1.0149x over previous
"""Trainium2 Bass kernel for the k-mer transformer problem.

Semantics (k=3, one-hot 3-mer filters over 4 bases):
    z[l, c] = relu(x[l,d0] + x[l+1,d1] + x[l+2,d2] - 2)
      where c = 16*d0 + 4*d1 + d2,  l in [0, 99999)
    out[b, 0, r*33333 + q, c] = z[3q + r, c]      (mod-3 interleave)

Strategy: pure data parallel (batch elem b -> NeuronCore b), and the conv
is a matmul on the tensor engine (the only engine with headroom: vector
f32 adds cap at ~123 G elem/s, Pool's software tensor_tensor at ~40-65).

Key layout trick: within one phase r the positions l = 3q + r are stride-3,
so each output position consumes 12 *consecutive* x floats
x.flat[12q+4r : 12q+4r+12] -- no input replication. Two position streams
(A: q in [0,16704), B: q+16704) are packed as M=128 output rows
(64 channels x 2 streams), so the PE streams 2 positions per column.
The staged moving tensor holds one 24-row window per phase: row 8t+s of
phase r's window is x.flat[12m + 4(r+t) + s] for s<4 (stream A) and
x.flat[12(m+16704) + 4(r+t) + s-4] for s>=4 (stream B). Matmul operands
must sit at SBUF base partition 0/32/64, and DMA into few partitions is
slow (per-partition SBUF write bandwidth), so the windows are spread over
partitions: xtA [56, M+128] carries phase 0 at rows 0:24 and phase 1 at
rows 32:56 (plus the stationary W[24,128] in its last 128 columns), and
xtB [24, M] carries phase 2 at rows 0:24. A single W serves all phases
(the window structure is phase-invariant); the -2 bias folds into the
relu evict, not the matmul.

PSUM [128, 512] f32 tiles (one bank per matmul, 4 banks per group, 2
groups in flight) are evicted as relu(v-2) -> bf16 by ACT / Pool / DVE in
parallel column slices sized to their measured rates (~153/92/57 G
elem/s). Output rides to HBM as bf16 (12.8 MB/core; harness tolerance
2e-2 dwarfs the ~8e-3 worst-case bf16 path error); the host transposes
[2*64, 16704] -> [q, c] and upcasts during the gather.

DMA budget: 1 XT load + 1 W load + 2 stores x 3 phases = 8 <= 8 HWDGE sem
lanes (9+ adds a lane-reuse wait -> walrus "Too many sync wait commands").
"""

import sys

import ml_dtypes
import numpy as np

sys.path.insert(0, "/opt/trn_rl_repo")

import concourse.bacc as bacc  # noqa: E402
import concourse.mybir as mybir  # noqa: E402
from concourse.bass_utils import run_bass_kernel_spmd  # noqa: E402
from concourse.tile import TileContext  # noqa: E402

P = 128
Q = 33333  # valid q-positions per phase (99999 / 3)
M = 16704  # columns per stream (2 streams: q and q + M; 2*M = 33408 >= Q)
NROW = 24  # XT rows: 3 t-blocks x (4 A-rows + 4 B-rows), one window/phase
GROUP = 1024  # psum group columns (2 banks x 512); 4 groups in flight
# evict column split per group, proportional to measured engine rates
# (GPSIMD/Pool cannot read PSUM; ACT ~1.03 GHz and DVE ~0.89 GHz effective)
ACT_COLS = 608  # DVE gets the rest (416)
STORE_A = 10 * GROUP  # uneven store split so the trailing store is short
# Load descriptors above ~13KB run at half DMA-engine rate, and in-place
# splits coalesce back into one big descriptor. So each staged row is four
# 4416-element segments separated by 64-element SBUF gaps (non-adjacent ->
# no coalescing; ~8.8KB descriptors run at full rate). Segment k holds data
# columns [4096k, 4096(k+1)) (last: ..16704), with W in the first 128
# elements of segment 0 (so data starts at offset 128 there). 4096 is a
# psum-group multiple, so no matmul slice crosses a segment boundary.
SEG = 4416
SEGSTRIDE = SEG + 64  # 4480
NSEG = 4
ROWW = NSEG * SEGSTRIDE  # 17920 elements per staged SBUF row


def _phys(d):
    """Physical column of logical data column d."""
    k = min(d // 4096, NSEG - 1)
    return k * SEGSTRIDE + (128 if k == 0 else 0) + (d - 4096 * k)
L = 100001
N_CORES = 8

_CACHE = {}


def _kmer_w():
    """Stationary [24, 128] weights: row 8*jj+s, s<4 -> tap (jj, d=s) of
    stream A (cols 0:64), s>=4 -> tap (jj, d=s-4) of stream B (cols 64:128).
    """
    w = np.zeros((24, 128), dtype=np.float32)
    c = np.arange(64)
    digits = np.stack([c // 16, (c // 4) % 4, c % 4])  # [jj, c]
    for k in range(24):
        jj, s = k // 8, k % 8
        blk, d = (0, s) if s < 4 else (1, s - 4)
        w[k, 64 * blk + c[digits[jj] == d]] = 1.0
    return w.astype(ml_dtypes.bfloat16)


def _build_bass():
    # Bacc (not raw Bass): its finalize() runs generate_event_semaphores,
    # which splits multi-sem waits (HW allows at most 1 wait per inst).
    nc = bacc.Bacc()
    f32 = mybir.dt.float32
    bf16 = mybir.dt.bfloat16
    add = mybir.AluOpType.add
    mx = mybir.AluOpType.max
    relu = mybir.ActivationFunctionType.Relu

    xta_d = nc.declare_dram_parameter("xta", [56, NSEG, SEG], bf16, isOutput=False)
    xtb_d = nc.declare_dram_parameter("xtb", [24, NSEG, SEG], bf16, isOutput=False)
    y_d = nc.declare_dram_parameter("y", [3, P, M], bf16, isOutput=True)

    with TileContext(nc) as tc:
        with (
            tc.tile_pool(name="xp", bufs=1) as xp,
            tc.tile_pool(name="pp", bufs=4, space="PSUM") as pp,
            tc.tile_pool(name="op_", bufs=2) as op_,
        ):
            xta_sb = xp.tile([56, ROWW], bf16)
            nc.sync.dma_start(
                out=xta_sb.rearrange("p (c g) -> p c g", c=NSEG)[:, :, 0:SEG],
                in_=xta_d[:],
            )
            xtb_sb = xp.tile([24, ROWW], bf16, tag="xtb")
            nc.sync.dma_start(
                out=xtb_sb.rearrange("p (c g) -> p c g", c=NSEG)[:, :, 0:SEG],
                in_=xtb_d[:],
            )
            bias_sb = xp.tile([P, 1], f32, tag="bias")
            nc.vector.memset(bias_sb, -2.0)
            for r in range(3):
                o = op_.tile([P, M], bf16, tag="o")
                # lhsT and rhs must share a base partition (0/32/64)
                xt, p0 = (xta_sb, 32 * r) if r < 2 else (xtb_sb, 0)
                rhs_rows = xt[p0 : p0 + 24]
                w_ap = xta_sb[p0 : p0 + 24, 0:128] if r < 2 else xtb_sb[0:24, 0:128]
                for g0 in range(0, M, GROUP):
                    gw = min(GROUP, M - g0)
                    ps = pp.tile([P, GROUP], f32, tag="ps")
                    for k0 in range(0, gw, 512):
                        kw = min(512, gw - k0)
                        c0 = _phys(g0 + k0)
                        nc.tensor.matmul(
                            out=ps[:, k0 : k0 + kw],
                            lhsT=w_ap,
                            rhs=rhs_rows[:, c0 : c0 + kw],
                            start=True,
                            stop=True,
                        )
                    # relu(v - 2) -> bf16; ACT and DVE split the columns
                    sp = (ACT_COLS * gw) // GROUP
                    nc.scalar.activation(
                        o[:, g0 : g0 + sp],
                        ps[:, 0:sp],
                        relu,
                        bias=bias_sb,
                    )
                    nc.vector.tensor_scalar(
                        o[:, g0 + sp : g0 + gw],
                        ps[:, sp:gw],
                        -2.0,
                        0.0,
                        add,
                        mx,
                    )
                nc.sync.dma_start(out=y_d[r, :, 0:STORE_A], in_=o[:, 0:STORE_A])
                nc.sync.dma_start(out=y_d[r, :, STORE_A:M], in_=o[:, STORE_A:M])
    return nc


def _stage_inputs(x):
    """x: [8,1,L,4] f32 -> per-core {'xta': [56, M+128], 'xtb': [24, M]}."""
    w = _kmer_w()
    need = 12 * (2 * M - 1) + 28  # last col of the r=2 window reads up to here
    in_maps = []
    for b in range(x.shape[0]):
        xf = np.zeros(need, dtype=np.float32)
        xf[: L * 4] = x[b, 0].ravel()

        def band(r):
            out = np.empty((NROW, M), dtype=np.float32)
            for t in range(3):
                for s in range(4):
                    out[8 * t + s] = xf[4 * (r + t) + s :: 12][:M]
                    out[8 * t + s + 4] = xf[12 * M + 4 * (r + t) + s :: 12][:M]
            return out

        def segs(rows, data, wmat):
            """Pack data into NSEG padded segments (W leads segment 0)."""
            out = np.zeros((rows, NSEG, SEG), dtype=ml_dtypes.bfloat16)
            out[: wmat.shape[0], 0, 0:128] = wmat
            for k in range(NSEG):
                hi = 4096 * (k + 1) if k < NSEG - 1 else M
                chunk = data[:, 4096 * k : hi]
                off = 128 if k == 0 else 0
                out[: data.shape[0], k, off : off + chunk.shape[1]] = chunk
            return out

        b16 = ml_dtypes.bfloat16
        xta = np.zeros((56, NSEG, SEG), dtype=b16)
        xta[0:24] = segs(24, band(0).astype(b16), w)
        xta[32:56] = segs(24, band(1).astype(b16), w)
        xtb = segs(24, band(2).astype(b16), w)
        in_maps.append({"xta": xta, "xtb": xtb})
    return in_maps


def _gather_output(results):
    out = np.empty((len(results), 1, 3 * Q, 64), dtype=np.float32)
    for b, res in enumerate(results):
        y = np.asarray(res["y"]).astype(np.float32)  # [3, 128, M]
        for r in range(3):
            zr = y[r].reshape(2, 64, M).transpose(0, 2, 1).reshape(2 * M, 64)
            out[b, 0, r * Q : (r + 1) * Q, :] = zr[:Q]
    return out


def _built_and_finalized():
    if "nc" not in _CACHE:
        nc = _build_bass()
        # run_bass_via_pjrt never finalizes; Bacc.finalize runs the register
        # allocation + sync-wait legalization passes walrus requires.
        nc.finalize()
        _CACHE["nc"] = nc
    return _CACHE["nc"]


def run(x, trace=False):
    nc = _built_and_finalized()
    in_maps = _stage_inputs(np.asarray(x, dtype=np.float32))
    bkr = run_bass_kernel_spmd(nc, in_maps, list(range(N_CORES)), trace=trace)
    return _gather_output(bkr.results), bkr


def kernel(x, W=None):
    out, _ = run(x, trace=False)
    return out
